# revision 9
# baseline (speedup 1.0000x reference)
"""NNConv+GRU message-passing network (ConvGRU) on 8 Trainium2 NeuronCores.

Strategy (per spec sharding hint, adapted):
  - Edges sharded by OWNER OF DST node (8 node ranges of 1024). Each core
    computes edge MLP + per-edge matvec + scatter-add locally for its nodes
    (scatter realized as matmul against a static 0/1 selection matrix).
  - Node state h (fp16) is node-sharded for the GRU and AllGathered once per
    conv layer so every core can gather h[src] for its edges (indirect DMA).
  - Per-edge weights We = reshape(MLP(edge_attr)) never touch DRAM: PE
    computes We tiles into PSUM, ACT evacuates to SBUF fp16, DVE does the
    per-edge matvec as one broadcast-multiply + a 2x-rate pairwise add tree
    (tensor_reduce runs at 1x on TRN2 and is avoided entirely).
  - GRU gate GEMMs accumulate ih+hh contributions in PSUM; gate biases fold
    into the ACT activations. All elementwise GRU math in fp16 (2x DVE).
  - Pooling = matmul against a (1/cnt)-scaled one-hot matrix + 16KB
    AllReduce; output MLP replicated.

Self-contained: only needs numpy + the concourse/bass stack installed in the
container. All shapes hardcoded for this problem size.
"""
import numpy as np
from ml_dtypes import bfloat16 as bf16

DIM = 64
DEPTHS = 3
N_NODES = 8192
N_EDGES = 16384
N_GRAPHS = 64
NC = 8
NPC = N_NODES // NC   # 1024 nodes per core
P = 128

TRACE = False
LAST_EXEC_NS = None
LAST_RESULTS = None

_CACHE = {}


def _build(T, b2_zero, W):
    """Build the (shared) 8-core SPMD program. Per-core data arrives via inputs."""
    import concourse.mybir as mybir
    import concourse.tile as tile
    from concourse import bacc
    import concourse.bass as bass
    from concourse.masks import make_identity

    f32 = mybir.dt.float32
    f16 = mybir.dt.bfloat16
    i32 = mybir.dt.int32
    AF = mybir.ActivationFunctionType
    OP = mybir.AluOpType
    EP = T * P  # padded edge count per core
    OFFS = [min(max(64 * t - 224, 0), NPC - W) for t in range(T)] if W < NPC else [0] * T

    nc = bacc.Bacc("TRN2", target_bir_lowering=False, debug=False, num_devices=NC)

    def din(name, shape, dt=f32):
        return nc.dram_tensor(name, shape, dt, kind="ExternalInput")

    xT_d = din("xT", [40, NPC], f16)
    eaT_d = din("eaT", [10, EP], f16)
    srcx_d = din("srcidx", [P, T], i32)
    S_d = din("S", [P, T * W], f16)
    pS_d = din("poolS", [NPC, N_GRAPHS], f16)
    fc0_wT_d = din("fc0_wT", [40, 32], f16)
    fc0_b_d = din("fc0_b", [32, 1])
    g0_wihT_d = din("g0_wihT", [32, 192], f16)
    g0_brz_d = din("g0_brz", [128, 1])
    g0_bihn_d = din("g0_bihn", [64, 1])
    g0_bhhn_d = din("g0_bhhn", [64, 1])
    w2p_d = [din(f"w2p{d}", [128, 4096], f16) for d in range(DEPTHS)]
    m1wT_d = [din(f"m1wT{d}", [10, 128], f16) for d in range(DEPTHS)]
    m1b_d = [din(f"m1b{d}", [128, 1]) for d in range(DEPTHS)]
    root_d = [din(f"root{d}", [64, 64], f16) for d in range(DEPTHS)]
    convb_d = [din(f"convb{d}", [64, 1]) for d in range(DEPTHS)]
    wihT_d = [din(f"wihT{d}", [64, 192], f16) for d in range(DEPTHS)]
    whhT_d = [din(f"whhT{d}", [64, 192], f16) for d in range(DEPTHS)]
    brz_d = [din(f"brz{d}", [128, 1]) for d in range(DEPTHS)]
    bihn_d = [din(f"bihn{d}", [64, 1]) for d in range(DEPTHS)]
    bhhn_d = [din(f"bhhn{d}", [64, 1]) for d in range(DEPTHS)]
    b2bc_d = None if b2_zero else [din(f"b2bc{d}", [128, 4096], f16) for d in range(DEPTHS)]
    o0wT_d = din("o0wT", [64, 64])
    o0b_d = din("o0b", [64, 1])
    o1wT_d = din("o1wT", [64, 32])
    o1b_d = din("o1b", [32, 1])
    o2wT_d = din("o2wT", [32, 1])
    o2b_d = din("o2b", [1, 1])

    y_d = nc.dram_tensor("y", [1, N_GRAPHS], f32, kind="ExternalOutput")

    RG = [list(range(NC))]

    with tile.TileContext(nc) as tc:
        with (
            tc.tile_pool(name="const", bufs=1) as cp,
            tc.tile_pool(name="work", bufs=2) as wp,
            tc.tile_pool(name="edge", bufs=6) as ep,
            tc.tile_pool(name="edged", bufs=2) as ed,
            tc.tile_pool(name="pwe", bufs=2, space="PSUM") as pwe,
            tc.tile_pool(name="pagg", bufs=1, space="PSUM") as pagg,
            tc.tile_pool(name="ptp", bufs=2, space="PSUM") as ptp,
            tc.tile_pool(name="dram", bufs=1, space="DRAM") as dp,
        ):
            # ---------------- constants to SBUF ----------------
            def load(name, dram, shape, dt=f32, eng=None):
                t = cp.tile(shape, dt, name=name)
                (eng or nc.sync).dma_start(t[:], dram[:, :])
                return t

            xT = load("xT_s", xT_d, [40, NPC], f16)
            eaT = load("eaT_s", eaT_d, [10, EP], f16, eng=nc.scalar)
            srcx = load("srcx_s", srcx_d, [P, T], i32)
            S = load("S_s", S_d, [P, T * W], f16, eng=nc.scalar)
            pS = cp.tile([P, 8 * N_GRAPHS], f16, name="pS_s")
            for c in range(8):
                nc.scalar.dma_start(
                    pS[:, c * N_GRAPHS:(c + 1) * N_GRAPHS],
                    pS_d[c * P:(c + 1) * P, :],
                )
            fc0_wT = load("fc0_wT_s", fc0_wT_d, [40, 32], f16)
            fc0_b = load("fc0_b_s", fc0_b_d, [32, 1])
            g0_wihT = load("g0_wihT_s", g0_wihT_d, [32, 192], f16)
            g0_brz = load("g0_brz_s", g0_brz_d, [128, 1])
            g0_bihn = load("g0_bihn_s", g0_bihn_d, [64, 1])
            g0_bhhn = load("g0_bhhn_s", g0_bhhn_d, [64, 1])
            w2p = [load(f"w2p_s{d}", w2p_d[d], [128, 4096], f16, eng=nc.scalar) for d in range(DEPTHS)]
            m1wT = [load(f"m1wT_s{d}", m1wT_d[d], [10, 128], f16) for d in range(DEPTHS)]
            m1b = [load(f"m1b_s{d}", m1b_d[d], [128, 1]) for d in range(DEPTHS)]
            rootw = [load(f"root_s{d}", root_d[d], [64, 64], f16) for d in range(DEPTHS)]
            convb = [load(f"convb_s{d}", convb_d[d], [64, 1]) for d in range(DEPTHS)]
            wihT = [load(f"wihT_s{d}", wihT_d[d], [64, 192], f16) for d in range(DEPTHS)]
            whhT = [load(f"whhT_s{d}", whhT_d[d], [64, 192], f16) for d in range(DEPTHS)]
            brz = [load(f"brz_s{d}", brz_d[d], [128, 1]) for d in range(DEPTHS)]
            bihn = [load(f"bihn_s{d}", bihn_d[d], [64, 1]) for d in range(DEPTHS)]
            bhhn = [load(f"bhhn_s{d}", bhhn_d[d], [64, 1]) for d in range(DEPTHS)]
            b2bc = (
                None if b2_zero else
                [load(f"b2bc_s{d}", b2bc_d[d], [128, 4096], f16, eng=nc.scalar) for d in range(DEPTHS)]
            )
            o0wT = load("o0wT_s", o0wT_d, [64, 64])
            o0b = load("o0b_s", o0b_d, [64, 1])
            o1wT = load("o1wT_s", o1wT_d, [64, 32])
            o1b = load("o1b_s", o1b_d, [32, 1])
            o2wT = load("o2wT_s", o2wT_d, [32, 1])
            o2b = load("o2b_s", o2b_d, [1, 1])

            ident = cp.tile([P, P], f16, name="ident")
            make_identity(nc, ident[:])

            hown = [dp.tile([NPC, DIM], f16, name=f"hown{d}") for d in range(DEPTHS)]
            hfull = [dp.tile([N_NODES, DIM], f16, name=f"hfull{d}") for d in range(DEPTHS)]
            ar_in = dp.tile([DIM, N_GRAPHS], f32, name="ar_in")
            ar_out = dp.tile([DIM, N_GRAPHS], f32, name="ar_out")
            warm_in = dp.tile([1, 16], f32, name="warm_in")
            warm_out = dp.tile([1, 16], f32, name="warm_out")
            wz = cp.tile([1, 16], f32, name="wz")
            nc.vector.memset(wz[:], 0)
            nc.sync.dma_start(warm_in[:, :], wz[:])
            nc.gpsimd.collective_compute(
                "AllReduce", OP.add, replica_groups=RG,
                ins=[warm_in.opt()], outs=[warm_out.opt()],
            )

            # ---------------- helpers ----------------
            def gru_tail(d, rz, hnb, ginb, h_prev, tagp):
                """rz [128,1024] f16 post-sigmoid (r||z); hnb = gh_n + bhh_n (f16)
                (pre-multiplied by r when h_prev is None); ginb = gi_n + bih_n
                (f16). Returns new h_T [64,1024] f16."""
                z_s = wp.tile([64, NPC], f16, name=f"z_{tagp}", tag="gru_z")
                nc.sync.dma_start(z_s[:], rz[64:128, :])
                t1 = wp.tile([64, NPC], f16, name=f"t1_{tagp}", tag="gru_t1")
                if h_prev is None:
                    nc.vector.tensor_tensor(out=t1[:], in0=hnb[:], in1=ginb[:], op=OP.add)
                else:
                    nc.vector.tensor_tensor(out=t1[:], in0=rz[0:64, :], in1=hnb[:], op=OP.mult)
                    nc.vector.tensor_tensor(out=t1[:], in0=t1[:], in1=ginb[:], op=OP.add)
                nt = wp.tile([64, NPC], f16, name=f"nt_{tagp}", tag="gru_nt")
                nc.scalar.activation(nt[:], t1[:], AF.Tanh)
                hm = wp.tile([64, NPC], f16, name=f"hm_{tagp}", tag="gru_hm")
                if h_prev is None:
                    # h=0: h' = n - z*n
                    nc.vector.tensor_tensor(out=hm[:], in0=z_s[:], in1=nt[:], op=OP.mult)
                    hnew = wp.tile([64, NPC], f16, name=f"h_{tagp}", tag="hT")
                    nc.vector.tensor_tensor(out=hnew[:], in0=nt[:], in1=hm[:], op=OP.subtract)
                else:
                    nc.vector.tensor_tensor(out=hm[:], in0=h_prev[:], in1=nt[:], op=OP.subtract)
                    nc.vector.tensor_tensor(out=hm[:], in0=hm[:], in1=z_s[:], op=OP.mult)
                    hnew = wp.tile([64, NPC], f16, name=f"h_{tagp}", tag="hT")
                    nc.vector.tensor_tensor(out=hnew[:], in0=hm[:], in1=nt[:], op=OP.add)
                return hnew

            def h_to_node_major(h_T, tagp):
                """PE-transpose h_T [64,1024] f16 -> one [128, 8*64] node-major tile."""
                hcat = wp.tile([P, 8 * DIM], f16, name=f"hnm_{tagp}", tag="hnm")
                for c in range(8):
                    tp = ptp.tile([P, DIM], f16, name=f"tp_{tagp}_{c}", tag="tp")
                    nc.tensor.transpose(
                        out=tp[:], in_=h_T[:, c * P:(c + 1) * P], identity=ident[0:64, 0:64]
                    )
                    nc.scalar.activation(hcat[:, c * DIM:(c + 1) * DIM], tp[:], AF.Copy)
                return hcat


            def mm(out_ap_fn, lhsT_ap, rhs_ap_fn, n_total, start, stop):
                """matmuls in 512-wide chunks: out[:, s] = lhsT.T @ rhs[:, s]."""
                off = 0
                while off < n_total:
                    n = min(512, n_total - off)
                    nc.tensor.matmul(
                        out_ap_fn(off, n), lhsT_ap, rhs_ap_fn(off, n),
                        start=start, stop=stop,
                    )
                    off += n

            # ---------------- phase 0: fc0 + gru0 (h0 = 0) ----------------
            x0_ps = pwe.tile([P, NPC], f32, name="x0_ps", tag="pwe")
            mm(lambda o, n: x0_ps[0:32, o:o + n], fc0_wT[:],
               lambda o, n: xT[:, o:o + n], NPC, True, True)
            x0r = wp.tile([32, NPC], f16, name="x0r")
            nc.scalar.activation(x0r[:], x0_ps[0:32, :], AF.Relu, bias=fc0_b[:, 0:1])

            g0rz_ps = pwe.tile([P, NPC], f32, name="g0rz_ps", tag="pwe")
            mm(lambda o, n: g0rz_ps[0:128, o:o + n], g0_wihT[:, 0:128],
               lambda o, n: x0r[:, o:o + n], NPC, True, True)
            rz0 = wp.tile([P, NPC], f16, name="rz0", tag="gru_rz")
            nc.scalar.activation(rz0[:], g0rz_ps[0:128, :], AF.Sigmoid, bias=g0_brz[:, 0:1])

            g0n_ps = pwe.tile([P, NPC], f32, name="g0n_ps", tag="pwe")
            mm(lambda o, n: g0n_ps[0:64, o:o + n], g0_wihT[:, 128:192],
               lambda o, n: x0r[:, o:o + n], NPC, True, True)
            gin0 = wp.tile([64, NPC], f16, name="gin0", tag="gru_gin")
            nc.scalar.activation(gin0[:], g0n_ps[0:64, :], AF.Identity, bias=g0_bihn[:, 0:1])
            # hn = r * bhh_n  (h=0 so gh_n = bhh_n, broadcast per partition)
            hn0 = wp.tile([64, NPC], f16, name="hn0", tag="gru_hn")
            nc.vector.tensor_scalar_mul(hn0[:], rz0[0:64, :], g0_bhhn[:, 0:1])
            h_T = gru_tail(-1, rz0, hn0, gin0, None, "p0")

            hcat = h_to_node_major(h_T, "p0")
            nc.sync.dma_start(
                hown[0][:, :].rearrange("(c p) i -> p c i", p=P),
                hcat[:].rearrange("p (c i) -> p c i", i=DIM))
            nc.gpsimd.collective_compute(
                "AllGather", OP.bypass, replica_groups=RG,
                ins=[hown[0].opt()], outs=[hfull[0].opt()],
            )

            # ---------------- conv depths ----------------
            for d in range(DEPTHS):
                # edge-MLP hidden: hidT [128, EP] fp16 (k on partitions)
                hidT = wp.tile([P, EP], f16, name=f"hidT{d}", tag="hidT")
                off = 0
                while off < EP:
                    n = min(1024, EP - off)
                    hp = pwe.tile([P, NPC], f32, name=f"hid_ps{d}_{off}", tag="pwe")
                    mm(lambda o, nn, _b=off: hp[:, o:o + nn], m1wT[d][:],
                       lambda o, nn, _b=off: eaT[:, _b + o:_b + o + nn], n, True, True)
                    nc.scalar.activation(
                        hidT[:, off:off + n], hp[:, 0:n], AF.Relu, bias=m1b[d][:, 0:1]
                    )
                    off += n

                aggT = pagg.tile([64, NPC], f32, name=f"aggT{d}", tag="agg")
                # root term first: start=True initializes the full accumulator
                mm(lambda o, n: aggT[0:64, o:o + n], rootw[d][:],
                   lambda o, n: h_T[:, o:o + n], NPC, True, False)

                for t in range(T):
                    # gather h[src] for this tile's 128 edges (fp16 rows)
                    hsf = ep.tile([P, DIM], f16, name=f"hsf{d}_{t}", tag="hsf")
                    nc.gpsimd.indirect_dma_start(
                        out=hsf[:], out_offset=None,
                        in_=hfull[d][:, :],
                        in_offset=bass.IndirectOffsetOnAxis(ap=srcx[:, t:t + 1], axis=0),
                    )

                    # We tile: [128e, (o=64, i=64)] via 4 psum chunks of 1024
                    wsb = ep.tile([P, 4096], f16, name=f"wsb{d}_{t}", tag="wsb")
                    for q in range(4):
                        wps = pwe.tile([P, NPC], f32, name=f"we{d}_{t}_{q}", tag="pwe")
                        mm(lambda o, n, _q=q: wps[:, o:o + n],
                           hidT[:, t * P:(t + 1) * P],
                           lambda o, n, _q=q: w2p[d][:, _q * 1024 + o:_q * 1024 + o + n],
                           1024, True, True)
                        nc.scalar.activation(
                            wsb[:, q * 1024:(q + 1) * 1024], wps[:, :], AF.Copy
                        )
                    if b2bc is not None:
                        nc.vector.tensor_tensor(
                            out=wsb[:], in0=wsb[:], in1=b2bc[d][:, :], op=OP.add
                        )

                    # per-edge matvec: one 2x-rate broadcast multiply ...
                    prod = ed.tile([P, 4096], f16, name=f"prod{d}_{t}", tag="prod")
                    nc.vector.tensor_tensor(
                        out=prod[:].rearrange("p (o i) -> p o i", i=64),
                        in0=wsb[:].rearrange("p (o i) -> p o i", i=64),
                        in1=hsf[:, :].unsqueeze(1).to_broadcast([P, 64, 64]),
                        op=OP.mult,
                    )
                    # ... then a pairwise 2x-rate add tree over i (6 levels)
                    tA = ed.tile([P, 2048], f16, name=f"tA{d}_{t}", tag="tA")
                    tB = ed.tile([P, 1024], f16, name=f"tB{d}_{t}", tag="tB")
                    msgh = ep.tile([P, DIM], f16, name=f"msgh{d}_{t}", tag="msgh")

                    def radd(out_ap, a_ap, b_ap):
                        nc.vector.tensor_tensor(out=out_ap, in0=a_ap, in1=b_ap, op=OP.add)

                    pv = prod[:].rearrange("p (o i) -> p o i", i=64)
                    radd(tA[:].rearrange("p (o i) -> p o i", i=32),
                         pv[:, :, 0:32], pv[:, :, 32:64])
                    av = tA[:].rearrange("p (o i) -> p o i", i=32)
                    radd(tB[:].rearrange("p (o i) -> p o i", i=16),
                         av[:, :, 0:16], av[:, :, 16:32])
                    bv = tB[:].rearrange("p (o i) -> p o i", i=16)
                    radd(tA[:, 0:512].rearrange("p (o i) -> p o i", i=8),
                         bv[:, :, 0:8], bv[:, :, 8:16])
                    av2 = tA[:, 0:512].rearrange("p (o i) -> p o i", i=8)
                    radd(tB[:, 0:256].rearrange("p (o i) -> p o i", i=4),
                         av2[:, :, 0:4], av2[:, :, 4:8])
                    bv2 = tB[:, 0:256].rearrange("p (o i) -> p o i", i=4)
                    radd(tA[:, 0:128].rearrange("p (o i) -> p o i", i=2),
                         bv2[:, :, 0:2], bv2[:, :, 2:4])
                    av3 = tA[:, 0:128].rearrange("p (o i) -> p o i", i=2)
                    radd(msgh[:].rearrange("p (o i) -> p o i", i=1),
                         av3[:, :, 0:1], av3[:, :, 1:2])

                    # scatter-add into aggT via selection matmul (n=1024, fp16)
                    mm(lambda o, n, _t=t: aggT[0:64, OFFS[_t] + o:OFFS[_t] + o + n],
                       msgh[:],
                       lambda o, n, _t=t: S[:, _t * W + o:_t * W + o + n],
                       W, False, (t == T - 1))
                xc = wp.tile([64, NPC], f16, name=f"xc{d}", tag="xc")
                nc.scalar.activation(xc[:], aggT[0:64, :], AF.Relu, bias=convb[d][:, 0:1])

                # ---- GRU(xc, h): ih+hh accumulated in PSUM per gate group ----
                rz_ps = pwe.tile([P, NPC], f32, name=f"rzps{d}", tag="pwe")
                mm(lambda o, n: rz_ps[0:128, o:o + n], wihT[d][:, 0:128],
                   lambda o, n: xc[:, o:o + n], NPC, True, False)
                mm(lambda o, n: rz_ps[0:128, o:o + n], whhT[d][:, 0:128],
                   lambda o, n: h_T[:, o:o + n], NPC, False, True)
                rz = wp.tile([P, NPC], f16, name=f"rz{d}", tag="gru_rz")
                nc.scalar.activation(rz[:], rz_ps[0:128, :], AF.Sigmoid, bias=brz[d][:, 0:1])

                ghn_ps = pwe.tile([P, NPC], f32, name=f"ghn{d}", tag="pwe")
                mm(lambda o, n: ghn_ps[0:64, o:o + n], whhT[d][:, 128:192],
                   lambda o, n: h_T[:, o:o + n], NPC, True, True)
                hnb = wp.tile([64, NPC], f16, name=f"hnb{d}", tag="gru_hn")
                nc.scalar.activation(hnb[:], ghn_ps[0:64, :], AF.Identity, bias=bhhn[d][:, 0:1])

                gin_ps = pwe.tile([P, NPC], f32, name=f"gin{d}", tag="pwe")
                mm(lambda o, n: gin_ps[0:64, o:o + n], wihT[d][:, 128:192],
                   lambda o, n: xc[:, o:o + n], NPC, True, True)
                ginb = wp.tile([64, NPC], f16, name=f"ginb{d}", tag="gru_gin")
                nc.scalar.activation(ginb[:], gin_ps[0:64, :], AF.Identity, bias=bihn[d][:, 0:1])

                h_T = gru_tail(d, rz, hnb, ginb, h_T, f"d{d}")

                hcat = h_to_node_major(h_T, f"d{d}")
                if d < DEPTHS - 1:
                    nc.sync.dma_start(
                        hown[d + 1][:, :].rearrange("(c p) i -> p c i", p=P),
                        hcat[:].rearrange("p (c i) -> p c i", i=DIM))
                    nc.gpsimd.collective_compute(
                        "AllGather", OP.bypass, replica_groups=RG,
                        ins=[hown[d + 1].opt()], outs=[hfull[d + 1].opt()],
                    )
                else:
                    pooled_ps = ptp.tile([64, N_GRAPHS], f32, name="pooled_ps", tag="tp")
                    for c in range(8):
                        nc.tensor.matmul(
                            pooled_ps[0:64, :],
                            hcat[:, c * DIM:(c + 1) * DIM],
                            pS[:, c * N_GRAPHS:(c + 1) * N_GRAPHS],
                            start=(c == 0), stop=(c == 7),
                        )
                    pooled_sb = wp.tile([64, N_GRAPHS], f32, name="pooled_sb")
                    nc.scalar.activation(pooled_sb[:], pooled_ps[0:64, :], AF.Copy)
                    nc.sync.dma_start(ar_in[:, :], pooled_sb[:])

            # ---------------- pooling AllReduce + output MLP ----------------
            nc.gpsimd.collective_compute(
                "AllReduce", OP.add, replica_groups=RG,
                ins=[ar_in.opt()], outs=[ar_out.opt()],
            )
            pooled = wp.tile([64, N_GRAPHS], f32, name="pooled")
            nc.sync.dma_start(pooled[:], ar_out[:, :])

            m1_ps = ptp.tile([64, N_GRAPHS], f32, name="m1_ps", tag="tp")
            nc.tensor.matmul(m1_ps[0:64, :], o0wT[:], pooled[:], start=True, stop=True)
            m1r = wp.tile([64, N_GRAPHS], f32, name="m1r")
            nc.scalar.activation(m1r[:], m1_ps[0:64, :], AF.Relu, bias=o0b[:, 0:1])

            m2_ps = ptp.tile([64, N_GRAPHS], f32, name="m2_ps", tag="tp")
            nc.tensor.matmul(m2_ps[0:32, :], o1wT[:], m1r[:], start=True, stop=True)
            m2b = wp.tile([32, N_GRAPHS], f32, name="m2b")
            nc.scalar.activation(m2b[:], m2_ps[0:32, :], AF.Identity, bias=o1b[:, 0:1])

            m3_ps = ptp.tile([64, N_GRAPHS], f32, name="m3_ps", tag="tp")
            nc.tensor.matmul(m3_ps[0:1, :], o2wT[:], m2b[:], start=True, stop=True)
            ysb = wp.tile([1, N_GRAPHS], f32, name="ysb")
            nc.scalar.activation(ysb[:], m3_ps[0:1, :], AF.Identity, bias=o2b[:, 0:1])
            nc.sync.dma_start(y_d[:, :], ysb[:])

    nc.finalize()
    return nc


def _prep(inputs):
    """Host-side sharding + weight permutation. Returns (T, b2_zero, W, in_maps)."""
    g = lambda k: np.asarray(inputs[k])
    x = g("x").astype(np.float32)
    ea = g("edge_attr").astype(np.float32)
    ei = g("edge_index").astype(np.int64)
    batch = g("batch").astype(np.int64)
    src, dst = ei[0], ei[1]

    owner = dst // NPC
    core_ids = [np.nonzero(owner == c)[0] for c in range(NC)]
    # sort each core's edges by destination so a 128-edge tile scatters into
    # a narrow, statically-known node window
    core_ids = [ids[np.argsort(dst[ids], kind="stable")] for ids in core_ids]
    T = int(max((len(ids) + P - 1) // P for ids in core_ids))
    T = max(T, 1)
    EP = T * P

    W = 512
    offs = [min(max(64 * t - 224, 0), NPC - W) for t in range(T)]
    for c in range(NC):
        dl = dst[core_ids[c]] - c * NPC
        for t in range(T):
            seg = dl[t * P:(t + 1) * P]
            if len(seg) and (seg.min() < offs[t] or seg.max() >= offs[t] + W):
                W = NPC
                break
        if W == NPC:
            break
    if W == NPC:
        offs = [0] * T

    cnt = np.bincount(batch, minlength=N_GRAPHS).astype(np.float32)
    inv = 1.0 / np.maximum(cnt, 1.0)

    mlp2_b = g("mlp2_b").astype(np.float32)
    b2_zero = bool(np.all(mlp2_b == 0))

    # ---- shared weights
    shared = {
        "fc0_wT": g("fc0_w").astype(bf16).T.copy(),
        "fc0_b": g("fc0_b").astype(np.float32)[:, None],
        "g0_wihT": g("gru0_wih").astype(bf16).T.copy(),
        "g0_brz": (g("gru0_bih") + g("gru0_bhh")).astype(np.float32)[:128, None],
        "g0_bihn": g("gru0_bih").astype(np.float32)[128:, None],
        "g0_bhhn": g("gru0_bhh").astype(np.float32)[128:, None],
        "o0wT": g("out0_w").astype(np.float32).T.copy(),
        "o0b": g("out0_b").astype(np.float32)[:, None],
        "o1wT": g("out1_w").astype(np.float32).T.copy(),
        "o1b": g("out1_b").astype(np.float32)[:, None],
        "o2wT": g("out2_w").astype(np.float32).T.copy(),
        "o2b": g("out2_b").astype(np.float32)[:, None],
    }
    mlp1_w = g("mlp1_w").astype(np.float32)
    mlp1_b = g("mlp1_b").astype(np.float32)
    mlp2_w = g("mlp2_w").astype(np.float32)
    root_w = g("root_w").astype(np.float32)
    conv_b = g("conv_b").astype(np.float32)
    gru_wih = g("gru_wih").astype(np.float32)
    gru_whh = g("gru_whh").astype(np.float32)
    gru_bih = g("gru_bih").astype(np.float32)
    gru_bhh = g("gru_bhh").astype(np.float32)
    for d in range(DEPTHS):
        shared[f"w2p{d}"] = (
            mlp2_w[d].reshape(64, 64, 128).transpose(2, 1, 0).reshape(128, 4096)
        ).astype(bf16)
        shared[f"m1wT{d}"] = mlp1_w[d].T.astype(bf16).copy()
        shared[f"m1b{d}"] = mlp1_b[d][:, None].copy()
        shared[f"root{d}"] = root_w[d].astype(bf16).copy()
        shared[f"convb{d}"] = conv_b[d][:, None].copy()
        shared[f"wihT{d}"] = gru_wih[d].T.astype(bf16).copy()
        shared[f"whhT{d}"] = gru_whh[d].T.astype(bf16).copy()
        shared[f"brz{d}"] = (gru_bih[d] + gru_bhh[d])[:128, None].copy()
        shared[f"bihn{d}"] = gru_bih[d][128:, None].copy()
        shared[f"bhhn{d}"] = gru_bhh[d][128:, None].copy()
        if not b2_zero:
            b2p = mlp2_b[d].reshape(64, 64).T.reshape(4096)  # [(o,i)]
            shared[f"b2bc{d}"] = np.broadcast_to(
                b2p.astype(bf16), (P, 4096)
            ).copy()

    in_maps = []
    for c in range(NC):
        ids = core_ids[c]
        n_real = len(ids)
        src_pad = np.zeros(EP, np.int32)
        src_pad[:n_real] = src[ids]
        ea_pad = np.zeros((EP, 10), np.float32)
        ea_pad[:n_real] = ea[ids]
        dl = dst[ids] - c * NPC
        S_tab = np.zeros((P, T * W), bf16)
        for t in range(T):
            seg = dl[t * P:min((t + 1) * P, n_real)]
            S_tab[np.arange(len(seg)), t * W + seg - offs[t]] = 1.0
        pm = np.zeros((NPC, N_GRAPHS), bf16)
        nb = batch[c * NPC:(c + 1) * NPC]
        pm[np.arange(NPC), nb] = inv[nb].astype(bf16)
        m = {
            "xT": x[c * NPC:(c + 1) * NPC].T.astype(bf16).copy(),
            "eaT": ea_pad.T.astype(bf16).copy(),
            "srcidx": src_pad.reshape(T, P).T.copy(),
            "S": S_tab,
            "poolS": pm,
        }
        m.update(shared)
        in_maps.append(m)
    return T, b2_zero, W, in_maps


def kernel(**inputs) -> np.ndarray:
    global LAST_EXEC_NS, LAST_RESULTS
    T, b2_zero, W, in_maps = _prep(inputs)
    key = (T, b2_zero, W)
    if key not in _CACHE:
        _CACHE[key] = _build(T, b2_zero, W)
    nc = _CACHE[key]

    from concourse.bass_utils import run_bass_kernel_spmd

    if TRACE:
        res = run_bass_kernel_spmd(
            nc, in_maps, list(range(NC)), trace=True, trace_cores=list(range(NC))
        )
        LAST_EXEC_NS = res.exec_time_ns
        LAST_RESULTS = res
    else:
        res = run_bass_kernel_spmd(nc, in_maps, list(range(NC)))
    return res.results[0]["y"].reshape(N_GRAPHS).astype(np.float32)


# revision 10
# speedup vs baseline: 1.0054x; 1.0054x over previous
"""NNConv+GRU message-passing network (ConvGRU) on 8 Trainium2 NeuronCores.

Strategy (per spec sharding hint, adapted):
  - Edges sharded by OWNER OF DST node (8 node ranges of 1024). Each core
    computes edge MLP + per-edge matvec + scatter-add locally for its nodes
    (scatter realized as matmul against a static 0/1 selection matrix).
  - Node state h (fp16) is node-sharded for the GRU and AllGathered once per
    conv layer so every core can gather h[src] for its edges (indirect DMA).
  - Per-edge weights We = reshape(MLP(edge_attr)) never touch DRAM: PE
    computes We tiles into PSUM, ACT evacuates to SBUF fp16, DVE does the
    per-edge matvec as one broadcast-multiply + a 2x-rate pairwise add tree
    (tensor_reduce runs at 1x on TRN2 and is avoided entirely).
  - GRU gate GEMMs accumulate ih+hh contributions in PSUM; gate biases fold
    into the ACT activations. All elementwise GRU math in fp16 (2x DVE).
  - Pooling = matmul against a (1/cnt)-scaled one-hot matrix + 16KB
    AllReduce; output MLP replicated.

Self-contained: only needs numpy + the concourse/bass stack installed in the
container. All shapes hardcoded for this problem size.
"""
import numpy as np
from ml_dtypes import bfloat16 as bf16

DIM = 64
DEPTHS = 3
N_NODES = 8192
N_EDGES = 16384
N_GRAPHS = 64
NC = 8
NPC = N_NODES // NC   # 1024 nodes per core
P = 128

TRACE = False
LAST_EXEC_NS = None
LAST_RESULTS = None

_CACHE = {}


def _build(T, b2_zero, W):
    """Build the (shared) 8-core SPMD program. Per-core data arrives via inputs."""
    import concourse.mybir as mybir
    import concourse.tile as tile
    from concourse import bacc
    import concourse.bass as bass
    from concourse.masks import make_identity

    f32 = mybir.dt.float32
    f16 = mybir.dt.bfloat16
    i32 = mybir.dt.int32
    AF = mybir.ActivationFunctionType
    OP = mybir.AluOpType
    EP = T * P  # padded edge count per core
    OFFS = [min(max(64 * t - 224, 0), NPC - W) for t in range(T)] if W < NPC else [0] * T

    nc = bacc.Bacc("TRN2", target_bir_lowering=False, debug=False, num_devices=NC)

    def din(name, shape, dt=f32):
        return nc.dram_tensor(name, shape, dt, kind="ExternalInput")

    xT_d = din("xT", [40, NPC], f16)
    eaT_d = din("eaT", [10, EP], f16)
    srcx_d = din("srcidx", [P, T], i32)
    S_d = din("S", [P, T * W], f16)
    pS_d = din("poolS", [NPC, N_GRAPHS], f16)
    fc0_wT_d = din("fc0_wT", [40, 32], f16)
    fc0_b_d = din("fc0_b", [32, 1])
    g0_wihT_d = din("g0_wihT", [32, 192], f16)
    g0_brz_d = din("g0_brz", [128, 1])
    g0_bihn_d = din("g0_bihn", [64, 1])
    g0_bhhn_d = din("g0_bhhn", [64, 1])
    w2p_d = [din(f"w2p{d}", [128, 4096], f16) for d in range(DEPTHS)]
    m1wT_d = [din(f"m1wT{d}", [10, 128], f16) for d in range(DEPTHS)]
    m1b_d = [din(f"m1b{d}", [128, 1]) for d in range(DEPTHS)]
    root_d = [din(f"root{d}", [64, 64], f16) for d in range(DEPTHS)]
    convb_d = [din(f"convb{d}", [64, 1]) for d in range(DEPTHS)]
    wihT_d = [din(f"wihT{d}", [64, 192], f16) for d in range(DEPTHS)]
    whhT_d = [din(f"whhT{d}", [64, 192], f16) for d in range(DEPTHS)]
    brz_d = [din(f"brz{d}", [128, 1]) for d in range(DEPTHS)]
    bihn_d = [din(f"bihn{d}", [64, 1]) for d in range(DEPTHS)]
    bhhn_d = [din(f"bhhn{d}", [64, 1]) for d in range(DEPTHS)]
    b2bc_d = None if b2_zero else [din(f"b2bc{d}", [128, 4096], f16) for d in range(DEPTHS)]
    o0wT_d = din("o0wT", [64, 64])
    o0b_d = din("o0b", [64, 1])
    o1wT_d = din("o1wT", [64, 32])
    o1b_d = din("o1b", [32, 1])
    o2wT_d = din("o2wT", [32, 1])
    o2b_d = din("o2b", [1, 1])

    y_d = nc.dram_tensor("y", [1, N_GRAPHS], f32, kind="ExternalOutput")

    RG = [list(range(NC))]

    with tile.TileContext(nc) as tc:
        with (
            tc.tile_pool(name="const", bufs=1) as cp,
            tc.tile_pool(name="work", bufs=2) as wp,
            tc.tile_pool(name="edge", bufs=8) as ep,
            tc.tile_pool(name="edged", bufs=2) as ed,
            tc.tile_pool(name="pwe", bufs=2, space="PSUM") as pwe,
            tc.tile_pool(name="pagg", bufs=1, space="PSUM") as pagg,
            tc.tile_pool(name="ptp", bufs=2, space="PSUM") as ptp,
            tc.tile_pool(name="dram", bufs=1, space="DRAM") as dp,
        ):
            # ---------------- constants to SBUF ----------------
            def load(name, dram, shape, dt=f32, eng=None):
                t = cp.tile(shape, dt, name=name)
                (eng or nc.sync).dma_start(t[:], dram[:, :])
                return t

            xT = load("xT_s", xT_d, [40, NPC], f16)
            eaT = load("eaT_s", eaT_d, [10, EP], f16, eng=nc.gpsimd)
            srcx = load("srcx_s", srcx_d, [P, T], i32)
            S = load("S_s", S_d, [P, T * W], f16, eng=nc.gpsimd)
            pS = cp.tile([P, 8 * N_GRAPHS], f16, name="pS_s")
            for c in range(8):
                nc.gpsimd.dma_start(
                    pS[:, c * N_GRAPHS:(c + 1) * N_GRAPHS],
                    pS_d[c * P:(c + 1) * P, :],
                )
            fc0_wT = load("fc0_wT_s", fc0_wT_d, [40, 32], f16)
            fc0_b = load("fc0_b_s", fc0_b_d, [32, 1])
            g0_wihT = load("g0_wihT_s", g0_wihT_d, [32, 192], f16)
            g0_brz = load("g0_brz_s", g0_brz_d, [128, 1])
            g0_bihn = load("g0_bihn_s", g0_bihn_d, [64, 1])
            g0_bhhn = load("g0_bhhn_s", g0_bhhn_d, [64, 1])
            w2p = [load(f"w2p_s{d}", w2p_d[d], [128, 4096], f16, eng=nc.gpsimd) for d in range(DEPTHS)]
            m1wT = [load(f"m1wT_s{d}", m1wT_d[d], [10, 128], f16) for d in range(DEPTHS)]
            m1b = [load(f"m1b_s{d}", m1b_d[d], [128, 1]) for d in range(DEPTHS)]
            rootw = [load(f"root_s{d}", root_d[d], [64, 64], f16) for d in range(DEPTHS)]
            convb = [load(f"convb_s{d}", convb_d[d], [64, 1]) for d in range(DEPTHS)]
            wihT = [load(f"wihT_s{d}", wihT_d[d], [64, 192], f16) for d in range(DEPTHS)]
            whhT = [load(f"whhT_s{d}", whhT_d[d], [64, 192], f16) for d in range(DEPTHS)]
            brz = [load(f"brz_s{d}", brz_d[d], [128, 1]) for d in range(DEPTHS)]
            bihn = [load(f"bihn_s{d}", bihn_d[d], [64, 1]) for d in range(DEPTHS)]
            bhhn = [load(f"bhhn_s{d}", bhhn_d[d], [64, 1]) for d in range(DEPTHS)]
            b2bc = (
                None if b2_zero else
                [load(f"b2bc_s{d}", b2bc_d[d], [128, 4096], f16, eng=nc.gpsimd) for d in range(DEPTHS)]
            )
            o0wT = load("o0wT_s", o0wT_d, [64, 64])
            o0b = load("o0b_s", o0b_d, [64, 1])
            o1wT = load("o1wT_s", o1wT_d, [64, 32])
            o1b = load("o1b_s", o1b_d, [32, 1])
            o2wT = load("o2wT_s", o2wT_d, [32, 1])
            o2b = load("o2b_s", o2b_d, [1, 1])

            ident = cp.tile([P, P], f16, name="ident")
            make_identity(nc, ident[:])

            hown = [dp.tile([NPC, DIM], f16, name=f"hown{d}") for d in range(DEPTHS)]
            hfull = [dp.tile([N_NODES, DIM], f16, name=f"hfull{d}") for d in range(DEPTHS)]
            ar_in = dp.tile([DIM, N_GRAPHS], f32, name="ar_in")
            ar_out = dp.tile([DIM, N_GRAPHS], f32, name="ar_out")
            warm_in = dp.tile([1, 16], f32, name="warm_in")
            warm_out = dp.tile([1, 16], f32, name="warm_out")
            wz = cp.tile([1, 16], f32, name="wz")
            nc.vector.memset(wz[:], 0)
            nc.sync.dma_start(warm_in[:, :], wz[:])
            nc.gpsimd.collective_compute(
                "AllReduce", OP.add, replica_groups=RG,
                ins=[warm_in.opt()], outs=[warm_out.opt()],
            )

            # ---------------- helpers ----------------
            def gru_tail(d, rz, hnb, ginb, h_prev, tagp):
                """rz [128,1024] f16 post-sigmoid (r||z); hnb = gh_n + bhh_n (f16)
                (pre-multiplied by r when h_prev is None); ginb = gi_n + bih_n
                (f16). Returns new h_T [64,1024] f16."""
                z_s = wp.tile([64, NPC], f16, name=f"z_{tagp}", tag="gru_z")
                nc.sync.dma_start(z_s[:], rz[64:128, :])
                t1 = wp.tile([64, NPC], f16, name=f"t1_{tagp}", tag="gru_t1")
                if h_prev is None:
                    nc.vector.tensor_tensor(out=t1[:], in0=hnb[:], in1=ginb[:], op=OP.add)
                else:
                    nc.vector.tensor_tensor(out=t1[:], in0=rz[0:64, :], in1=hnb[:], op=OP.mult)
                    nc.vector.tensor_tensor(out=t1[:], in0=t1[:], in1=ginb[:], op=OP.add)
                nt = wp.tile([64, NPC], f16, name=f"nt_{tagp}", tag="gru_nt")
                nc.scalar.activation(nt[:], t1[:], AF.Tanh)
                hm = wp.tile([64, NPC], f16, name=f"hm_{tagp}", tag="gru_hm")
                if h_prev is None:
                    # h=0: h' = n - z*n
                    nc.vector.tensor_tensor(out=hm[:], in0=z_s[:], in1=nt[:], op=OP.mult)
                    hnew = wp.tile([64, NPC], f16, name=f"h_{tagp}", tag="hT")
                    nc.vector.tensor_tensor(out=hnew[:], in0=nt[:], in1=hm[:], op=OP.subtract)
                else:
                    nc.vector.tensor_tensor(out=hm[:], in0=h_prev[:], in1=nt[:], op=OP.subtract)
                    nc.vector.tensor_tensor(out=hm[:], in0=hm[:], in1=z_s[:], op=OP.mult)
                    hnew = wp.tile([64, NPC], f16, name=f"h_{tagp}", tag="hT")
                    nc.vector.tensor_tensor(out=hnew[:], in0=hm[:], in1=nt[:], op=OP.add)
                return hnew

            def h_to_node_major(h_T, tagp):
                """PE-transpose h_T [64,1024] f16 -> one [128, 8*64] node-major tile."""
                hcat = wp.tile([P, 8 * DIM], f16, name=f"hnm_{tagp}", tag="hnm")
                for c in range(8):
                    tp = ptp.tile([P, DIM], f16, name=f"tp_{tagp}_{c}", tag="tp")
                    nc.tensor.transpose(
                        out=tp[:], in_=h_T[:, c * P:(c + 1) * P], identity=ident[0:64, 0:64]
                    )
                    nc.scalar.activation(hcat[:, c * DIM:(c + 1) * DIM], tp[:], AF.Copy)
                return hcat


            def mm(out_ap_fn, lhsT_ap, rhs_ap_fn, n_total, start, stop):
                """matmuls in 512-wide chunks: out[:, s] = lhsT.T @ rhs[:, s]."""
                off = 0
                while off < n_total:
                    n = min(512, n_total - off)
                    nc.tensor.matmul(
                        out_ap_fn(off, n), lhsT_ap, rhs_ap_fn(off, n),
                        start=start, stop=stop,
                    )
                    off += n

            # ---------------- phase 0: fc0 + gru0 (h0 = 0) ----------------
            x0_ps = pwe.tile([P, NPC], f32, name="x0_ps", tag="pwe")
            mm(lambda o, n: x0_ps[0:32, o:o + n], fc0_wT[:],
               lambda o, n: xT[:, o:o + n], NPC, True, True)
            x0r = wp.tile([32, NPC], f16, name="x0r")
            nc.scalar.activation(x0r[:], x0_ps[0:32, :], AF.Relu, bias=fc0_b[:, 0:1])

            g0rz_ps = pwe.tile([P, NPC], f32, name="g0rz_ps", tag="pwe")
            mm(lambda o, n: g0rz_ps[0:128, o:o + n], g0_wihT[:, 0:128],
               lambda o, n: x0r[:, o:o + n], NPC, True, True)
            rz0 = wp.tile([P, NPC], f16, name="rz0", tag="gru_rz")
            nc.scalar.activation(rz0[:], g0rz_ps[0:128, :], AF.Sigmoid, bias=g0_brz[:, 0:1])

            g0n_ps = pwe.tile([P, NPC], f32, name="g0n_ps", tag="pwe")
            mm(lambda o, n: g0n_ps[0:64, o:o + n], g0_wihT[:, 128:192],
               lambda o, n: x0r[:, o:o + n], NPC, True, True)
            gin0 = wp.tile([64, NPC], f16, name="gin0", tag="gru_gin")
            nc.scalar.activation(gin0[:], g0n_ps[0:64, :], AF.Identity, bias=g0_bihn[:, 0:1])
            # hn = r * bhh_n  (h=0 so gh_n = bhh_n, broadcast per partition)
            hn0 = wp.tile([64, NPC], f16, name="hn0", tag="gru_hn")
            nc.vector.tensor_scalar_mul(hn0[:], rz0[0:64, :], g0_bhhn[:, 0:1])
            h_T = gru_tail(-1, rz0, hn0, gin0, None, "p0")

            hcat = h_to_node_major(h_T, "p0")
            nc.sync.dma_start(
                hown[0][:, :].rearrange("(c p) i -> p c i", p=P),
                hcat[:].rearrange("p (c i) -> p c i", i=DIM))
            nc.gpsimd.collective_compute(
                "AllGather", OP.bypass, replica_groups=RG,
                ins=[hown[0].opt()], outs=[hfull[0].opt()],
            )

            # ---------------- conv depths ----------------
            for d in range(DEPTHS):
                # edge-MLP hidden: hidT [128, EP] fp16 (k on partitions)
                hidT = wp.tile([P, EP], f16, name=f"hidT{d}", tag="hidT")
                off = 0
                while off < EP:
                    n = min(1024, EP - off)
                    hp = pwe.tile([P, NPC], f32, name=f"hid_ps{d}_{off}", tag="pwe")
                    mm(lambda o, nn, _b=off: hp[:, o:o + nn], m1wT[d][:],
                       lambda o, nn, _b=off: eaT[:, _b + o:_b + o + nn], n, True, True)
                    nc.scalar.activation(
                        hidT[:, off:off + n], hp[:, 0:n], AF.Relu, bias=m1b[d][:, 0:1]
                    )
                    off += n

                aggT = pagg.tile([64, NPC], f32, name=f"aggT{d}", tag="agg")
                # root term first: start=True initializes the full accumulator
                mm(lambda o, n: aggT[0:64, o:o + n], rootw[d][:],
                   lambda o, n: h_T[:, o:o + n], NPC, True, False)

                for t in range(T):
                    # gather h[src] for this tile's 128 edges (fp16 rows)
                    hsf = ep.tile([P, DIM], f16, name=f"hsf{d}_{t}", tag="hsf")
                    nc.gpsimd.indirect_dma_start(
                        out=hsf[:], out_offset=None,
                        in_=hfull[d][:, :],
                        in_offset=bass.IndirectOffsetOnAxis(ap=srcx[:, t:t + 1], axis=0),
                    )

                    # We tile: [128e, (o=64, i=64)] via 4 psum chunks of 1024
                    wsb = ep.tile([P, 4096], f16, name=f"wsb{d}_{t}", tag="wsb")
                    for q in range(4):
                        wps = pwe.tile([P, NPC], f32, name=f"we{d}_{t}_{q}", tag="pwe")
                        mm(lambda o, n, _q=q: wps[:, o:o + n],
                           hidT[:, t * P:(t + 1) * P],
                           lambda o, n, _q=q: w2p[d][:, _q * 1024 + o:_q * 1024 + o + n],
                           1024, True, True)
                        nc.scalar.activation(
                            wsb[:, q * 1024:(q + 1) * 1024], wps[:, :], AF.Copy
                        )
                    if b2bc is not None:
                        nc.vector.tensor_tensor(
                            out=wsb[:], in0=wsb[:], in1=b2bc[d][:, :], op=OP.add
                        )

                    # per-edge matvec: one 2x-rate broadcast multiply ...
                    prod = ed.tile([P, 4096], f16, name=f"prod{d}_{t}", tag="prod")
                    nc.vector.tensor_tensor(
                        out=prod[:].rearrange("p (o i) -> p o i", i=64),
                        in0=wsb[:].rearrange("p (o i) -> p o i", i=64),
                        in1=hsf[:, :].unsqueeze(1).to_broadcast([P, 64, 64]),
                        op=OP.mult,
                    )
                    # ... then a pairwise 2x-rate add tree over i (6 levels)
                    tA = ed.tile([P, 2048], f16, name=f"tA{d}_{t}", tag="tA")
                    tB = ed.tile([P, 1024], f16, name=f"tB{d}_{t}", tag="tB")
                    msgh = ep.tile([P, DIM], f16, name=f"msgh{d}_{t}", tag="msgh")

                    def radd(out_ap, a_ap, b_ap):
                        nc.vector.tensor_tensor(out=out_ap, in0=a_ap, in1=b_ap, op=OP.add)

                    pv = prod[:].rearrange("p (o i) -> p o i", i=64)
                    radd(tA[:].rearrange("p (o i) -> p o i", i=32),
                         pv[:, :, 0:32], pv[:, :, 32:64])
                    av = tA[:].rearrange("p (o i) -> p o i", i=32)
                    radd(tB[:].rearrange("p (o i) -> p o i", i=16),
                         av[:, :, 0:16], av[:, :, 16:32])
                    bv = tB[:].rearrange("p (o i) -> p o i", i=16)
                    radd(tA[:, 0:512].rearrange("p (o i) -> p o i", i=8),
                         bv[:, :, 0:8], bv[:, :, 8:16])
                    av2 = tA[:, 0:512].rearrange("p (o i) -> p o i", i=8)
                    radd(tB[:, 0:256].rearrange("p (o i) -> p o i", i=4),
                         av2[:, :, 0:4], av2[:, :, 4:8])
                    bv2 = tB[:, 0:256].rearrange("p (o i) -> p o i", i=4)
                    radd(tA[:, 0:128].rearrange("p (o i) -> p o i", i=2),
                         bv2[:, :, 0:2], bv2[:, :, 2:4])
                    av3 = tA[:, 0:128].rearrange("p (o i) -> p o i", i=2)
                    radd(msgh[:].rearrange("p (o i) -> p o i", i=1),
                         av3[:, :, 0:1], av3[:, :, 1:2])

                    # scatter-add into aggT via selection matmul (n=1024, fp16)
                    mm(lambda o, n, _t=t: aggT[0:64, OFFS[_t] + o:OFFS[_t] + o + n],
                       msgh[:],
                       lambda o, n, _t=t: S[:, _t * W + o:_t * W + o + n],
                       W, False, (t == T - 1))
                xc = wp.tile([64, NPC], f16, name=f"xc{d}", tag="xc")
                nc.scalar.activation(xc[:], aggT[0:64, :], AF.Relu, bias=convb[d][:, 0:1])

                # ---- GRU(xc, h): ih+hh accumulated in PSUM per gate group ----
                rz_ps = pwe.tile([P, NPC], f32, name=f"rzps{d}", tag="pwe")
                mm(lambda o, n: rz_ps[0:128, o:o + n], wihT[d][:, 0:128],
                   lambda o, n: xc[:, o:o + n], NPC, True, False)
                mm(lambda o, n: rz_ps[0:128, o:o + n], whhT[d][:, 0:128],
                   lambda o, n: h_T[:, o:o + n], NPC, False, True)
                rz = wp.tile([P, NPC], f16, name=f"rz{d}", tag="gru_rz")
                nc.scalar.activation(rz[:], rz_ps[0:128, :], AF.Sigmoid, bias=brz[d][:, 0:1])

                ghn_ps = pwe.tile([P, NPC], f32, name=f"ghn{d}", tag="pwe")
                mm(lambda o, n: ghn_ps[0:64, o:o + n], whhT[d][:, 128:192],
                   lambda o, n: h_T[:, o:o + n], NPC, True, True)
                hnb = wp.tile([64, NPC], f16, name=f"hnb{d}", tag="gru_hn")
                nc.scalar.activation(hnb[:], ghn_ps[0:64, :], AF.Identity, bias=bhhn[d][:, 0:1])

                gin_ps = pwe.tile([P, NPC], f32, name=f"gin{d}", tag="pwe")
                mm(lambda o, n: gin_ps[0:64, o:o + n], wihT[d][:, 128:192],
                   lambda o, n: xc[:, o:o + n], NPC, True, True)
                ginb = wp.tile([64, NPC], f16, name=f"ginb{d}", tag="gru_gin")
                nc.scalar.activation(ginb[:], gin_ps[0:64, :], AF.Identity, bias=bihn[d][:, 0:1])

                h_T = gru_tail(d, rz, hnb, ginb, h_T, f"d{d}")

                hcat = h_to_node_major(h_T, f"d{d}")
                if d < DEPTHS - 1:
                    nc.sync.dma_start(
                        hown[d + 1][:, :].rearrange("(c p) i -> p c i", p=P),
                        hcat[:].rearrange("p (c i) -> p c i", i=DIM))
                    nc.gpsimd.collective_compute(
                        "AllGather", OP.bypass, replica_groups=RG,
                        ins=[hown[d + 1].opt()], outs=[hfull[d + 1].opt()],
                    )
                else:
                    pooled_ps = ptp.tile([64, N_GRAPHS], f32, name="pooled_ps", tag="tp")
                    for c in range(8):
                        nc.tensor.matmul(
                            pooled_ps[0:64, :],
                            hcat[:, c * DIM:(c + 1) * DIM],
                            pS[:, c * N_GRAPHS:(c + 1) * N_GRAPHS],
                            start=(c == 0), stop=(c == 7),
                        )
                    pooled_sb = wp.tile([64, N_GRAPHS], f32, name="pooled_sb")
                    nc.scalar.activation(pooled_sb[:], pooled_ps[0:64, :], AF.Copy)
                    nc.sync.dma_start(ar_in[:, :], pooled_sb[:])

            # ---------------- pooling AllReduce + output MLP ----------------
            nc.gpsimd.collective_compute(
                "AllReduce", OP.add, replica_groups=RG,
                ins=[ar_in.opt()], outs=[ar_out.opt()],
            )
            pooled = wp.tile([64, N_GRAPHS], f32, name="pooled")
            nc.sync.dma_start(pooled[:], ar_out[:, :])

            m1_ps = ptp.tile([64, N_GRAPHS], f32, name="m1_ps", tag="tp")
            nc.tensor.matmul(m1_ps[0:64, :], o0wT[:], pooled[:], start=True, stop=True)
            m1r = wp.tile([64, N_GRAPHS], f32, name="m1r")
            nc.scalar.activation(m1r[:], m1_ps[0:64, :], AF.Relu, bias=o0b[:, 0:1])

            m2_ps = ptp.tile([64, N_GRAPHS], f32, name="m2_ps", tag="tp")
            nc.tensor.matmul(m2_ps[0:32, :], o1wT[:], m1r[:], start=True, stop=True)
            m2b = wp.tile([32, N_GRAPHS], f32, name="m2b")
            nc.scalar.activation(m2b[:], m2_ps[0:32, :], AF.Identity, bias=o1b[:, 0:1])

            m3_ps = ptp.tile([64, N_GRAPHS], f32, name="m3_ps", tag="tp")
            nc.tensor.matmul(m3_ps[0:1, :], o2wT[:], m2b[:], start=True, stop=True)
            ysb = wp.tile([1, N_GRAPHS], f32, name="ysb")
            nc.scalar.activation(ysb[:], m3_ps[0:1, :], AF.Identity, bias=o2b[:, 0:1])
            nc.sync.dma_start(y_d[:, :], ysb[:])

    nc.finalize()
    return nc


def _prep(inputs):
    """Host-side sharding + weight permutation. Returns (T, b2_zero, W, in_maps)."""
    g = lambda k: np.asarray(inputs[k])
    x = g("x").astype(np.float32)
    ea = g("edge_attr").astype(np.float32)
    ei = g("edge_index").astype(np.int64)
    batch = g("batch").astype(np.int64)
    src, dst = ei[0], ei[1]

    owner = dst // NPC
    core_ids = [np.nonzero(owner == c)[0] for c in range(NC)]
    # sort each core's edges by destination so a 128-edge tile scatters into
    # a narrow, statically-known node window
    core_ids = [ids[np.argsort(dst[ids], kind="stable")] for ids in core_ids]
    T = int(max((len(ids) + P - 1) // P for ids in core_ids))
    T = max(T, 1)
    EP = T * P

    W = 512
    offs = [min(max(64 * t - 224, 0), NPC - W) for t in range(T)]
    for c in range(NC):
        dl = dst[core_ids[c]] - c * NPC
        for t in range(T):
            seg = dl[t * P:(t + 1) * P]
            if len(seg) and (seg.min() < offs[t] or seg.max() >= offs[t] + W):
                W = NPC
                break
        if W == NPC:
            break
    if W == NPC:
        offs = [0] * T

    cnt = np.bincount(batch, minlength=N_GRAPHS).astype(np.float32)
    inv = 1.0 / np.maximum(cnt, 1.0)

    mlp2_b = g("mlp2_b").astype(np.float32)
    b2_zero = bool(np.all(mlp2_b == 0))

    # ---- shared weights
    shared = {
        "fc0_wT": g("fc0_w").astype(bf16).T.copy(),
        "fc0_b": g("fc0_b").astype(np.float32)[:, None],
        "g0_wihT": g("gru0_wih").astype(bf16).T.copy(),
        "g0_brz": (g("gru0_bih") + g("gru0_bhh")).astype(np.float32)[:128, None],
        "g0_bihn": g("gru0_bih").astype(np.float32)[128:, None],
        "g0_bhhn": g("gru0_bhh").astype(np.float32)[128:, None],
        "o0wT": g("out0_w").astype(np.float32).T.copy(),
        "o0b": g("out0_b").astype(np.float32)[:, None],
        "o1wT": g("out1_w").astype(np.float32).T.copy(),
        "o1b": g("out1_b").astype(np.float32)[:, None],
        "o2wT": g("out2_w").astype(np.float32).T.copy(),
        "o2b": g("out2_b").astype(np.float32)[:, None],
    }
    mlp1_w = g("mlp1_w").astype(np.float32)
    mlp1_b = g("mlp1_b").astype(np.float32)
    mlp2_w = g("mlp2_w").astype(np.float32)
    root_w = g("root_w").astype(np.float32)
    conv_b = g("conv_b").astype(np.float32)
    gru_wih = g("gru_wih").astype(np.float32)
    gru_whh = g("gru_whh").astype(np.float32)
    gru_bih = g("gru_bih").astype(np.float32)
    gru_bhh = g("gru_bhh").astype(np.float32)
    for d in range(DEPTHS):
        shared[f"w2p{d}"] = (
            mlp2_w[d].reshape(64, 64, 128).transpose(2, 1, 0).reshape(128, 4096)
        ).astype(bf16)
        shared[f"m1wT{d}"] = mlp1_w[d].T.astype(bf16).copy()
        shared[f"m1b{d}"] = mlp1_b[d][:, None].copy()
        shared[f"root{d}"] = root_w[d].astype(bf16).copy()
        shared[f"convb{d}"] = conv_b[d][:, None].copy()
        shared[f"wihT{d}"] = gru_wih[d].T.astype(bf16).copy()
        shared[f"whhT{d}"] = gru_whh[d].T.astype(bf16).copy()
        shared[f"brz{d}"] = (gru_bih[d] + gru_bhh[d])[:128, None].copy()
        shared[f"bihn{d}"] = gru_bih[d][128:, None].copy()
        shared[f"bhhn{d}"] = gru_bhh[d][128:, None].copy()
        if not b2_zero:
            b2p = mlp2_b[d].reshape(64, 64).T.reshape(4096)  # [(o,i)]
            shared[f"b2bc{d}"] = np.broadcast_to(
                b2p.astype(bf16), (P, 4096)
            ).copy()

    in_maps = []
    for c in range(NC):
        ids = core_ids[c]
        n_real = len(ids)
        src_pad = np.zeros(EP, np.int32)
        src_pad[:n_real] = src[ids]
        ea_pad = np.zeros((EP, 10), np.float32)
        ea_pad[:n_real] = ea[ids]
        dl = dst[ids] - c * NPC
        S_tab = np.zeros((P, T * W), bf16)
        for t in range(T):
            seg = dl[t * P:min((t + 1) * P, n_real)]
            S_tab[np.arange(len(seg)), t * W + seg - offs[t]] = 1.0
        pm = np.zeros((NPC, N_GRAPHS), bf16)
        nb = batch[c * NPC:(c + 1) * NPC]
        pm[np.arange(NPC), nb] = inv[nb].astype(bf16)
        m = {
            "xT": x[c * NPC:(c + 1) * NPC].T.astype(bf16).copy(),
            "eaT": ea_pad.T.astype(bf16).copy(),
            "srcidx": src_pad.reshape(T, P).T.copy(),
            "S": S_tab,
            "poolS": pm,
        }
        m.update(shared)
        in_maps.append(m)
    return T, b2_zero, W, in_maps


def kernel(**inputs) -> np.ndarray:
    global LAST_EXEC_NS, LAST_RESULTS
    T, b2_zero, W, in_maps = _prep(inputs)
    key = (T, b2_zero, W)
    if key not in _CACHE:
        _CACHE[key] = _build(T, b2_zero, W)
    nc = _CACHE[key]

    from concourse.bass_utils import run_bass_kernel_spmd

    if TRACE:
        res = run_bass_kernel_spmd(
            nc, in_maps, list(range(NC)), trace=True, trace_cores=list(range(NC))
        )
        LAST_EXEC_NS = res.exec_time_ns
        LAST_RESULTS = res
    else:
        res = run_bass_kernel_spmd(nc, in_maps, list(range(NC)))
    return res.results[0]["y"].reshape(N_GRAPHS).astype(np.float32)
